# revision 1
# baseline (speedup 1.0000x reference)
"""MoE-routing LoRA linear for Trainium2, SPMD over 8 NeuronCores.

out = x @ base_w.T + base_b + 2.0 * lora_out, where lora_out routes each
token through its top-2 (of 8) LoRA experts with renormalized softmax gates.

Strategy: data-parallel over tokens (1024 tokens/core), weights replicated.
All heavy FLOPs are fp32r matmuls (full PE rate). The per-expert LoRA is
algebraically dense: h = x @ A_cat.T ([T,256]); gated hg = h * gates[e];
lora_out = hg @ W2 ([256,4096]) which is fused into the base matmul as two
extra contraction chunks.

v2 (trace-driven): phase 1 was 212us for ~60us of PE work (HAM oscillation
from per-tile PE stalls on the softmax chain, 44us of duplicate fp32 x
copies, DMA-paced transposes). Fixes: router reads the fp32r xT buffer
through an fp32 bitcast view (fp32r SBUF bytes are raw fp32 bits; rounding
happens in the PE datapath) so the top-2 selection stays bit-exact without
a second copy; hgT transposes deferred to the end of phase 1 so the PE
never waits on the per-tile gate chain; router+loraA matmuls interleaved
per chunk; 1MB x DMAs; phase-2 weight stream pool opened before phase 1 so
its first tiles prefetch early.
"""

import numpy as np

P = 128
B, S, D, O, E, R = 4, 2048, 4096, 4096, 8, 32
T = B * S            # 8192 tokens
NCORES = 8
TC = T // NCORES     # 1024 tokens per core
TT = TC // P         # 8 token tiles per core
DC = D // P          # 32 contraction chunks for x
ER = E * R           # 256
HC = ER // P         # 2 contraction chunks for hg
KC = DC + HC         # 34 total contraction chunks in the fused matmul
ON = 512             # output tile width
OT = O // ON         # 8 output tiles
SCALING = 64.0 / 32.0

TRACE = False        # test harness sets kernel.TRACE = True for profiling
LAST_RESULT = None   # BassKernelResults of the last run (for exec_time_ns)

_compiled = None


def _build():
    import concourse.mybir as mybir
    import concourse.tile as tile
    from concourse import bacc
    from concourse.masks import make_identity

    f32 = mybir.dt.float32
    f32r = mybir.dt.float32r
    bf16 = mybir.dt.bfloat16
    X = mybir.AxisListType.X
    mult = mybir.AluOpType.mult
    is_ge = mybir.AluOpType.is_ge
    Exp = mybir.ActivationFunctionType.Exp

    nc = bacc.Bacc("TRN2", target_bir_lowering=False, debug=False,
                   num_devices=NCORES)

    xs = nc.dram_tensor("xs", [TC, D], f32, kind="ExternalInput").ap()
    # fp32r-declared DRAM weights: DMA straight into fp32r SBUF tiles is
    # accepted by the BIR verifier and bit-identical to a DVE rounding pass
    # (verified empirically on HW).
    wbig = nc.dram_tensor("wbig", [KC * P, O], f32r, kind="ExternalInput").ap()
    wcat = nc.dram_tensor("wcat", [D, ER], f32r, kind="ExternalInput").ap()
    # router weights kept fp32: exact logits so top-2 selection matches the
    # fp32 reference (fp32r noise flips near-tied experts otherwise)
    wrouter = nc.dram_tensor("wrouter", [D, E], f32, kind="ExternalInput").ap()
    bias = nc.dram_tensor("bias", [P, O], bf16, kind="ExternalInput").ap()
    cbias = nc.dram_tensor("cbias", [P, E], f32, kind="ExternalInput").ap()
    out = nc.dram_tensor("out", [TC, O], f32, kind="ExternalOutput").ap()

    NCAT = ER  # 256

    with tile.TileContext(nc) as tc:
        with (
            tc.tile_pool(name="persist", bufs=1) as persist,
            tc.tile_pool(name="consts", bufs=1) as consts,
            tc.tile_pool(name="wstream", bufs=6) as wst,
        ):
            # x transposed + rounded to fp32r for the heavy matmuls. The
            # fp32r cast rounds on write, so the router CANNOT read this
            # buffer (top-2 flips on near-tied experts, and the BIR verifier
            # rejects fp32-typed producers feeding fp32r matmuls). The
            # router instead reads small transient exact-fp32 copies (x32)
            # made on the scalar engine.
            xT = persist.tile([P, DC, TC], f32r)
            hgT = persist.tile([P, HC, TC], f32r)    # gated h transposed
            ident = consts.tile([P, P], f32)
            make_identity(nc, ident[:])
            identb = consts.tile([P, P], bf16)
            nc.vector.tensor_copy(identb[:], ident[:])
            cbias_sb = consts.tile([P, E], f32)
            nc.sync.dma_start(cbias_sb[:], cbias)
            wrouter_sb = consts.tile([P, DC, E], f32)
            nc.sync.dma_start(
                wrouter_sb[:], wrouter.rearrange("(kc p) n -> p kc n", p=P))
            negbig = consts.tile([P, E], f32)
            nc.vector.memset(negbig[:], -1e30)

            # PE warm-up: ~2.5us of dense matmuls so the HAM clock gate opens
            # (K=8/8, 2.4GHz) at the start of phase 1.
            with (
                tc.tile_pool(name="wu_pool", bufs=1) as wupl,
                tc.tile_pool(name="wu_psum", bufs=1, space="PSUM") as wup,
            ):
                wu = wupl.tile([P, 512], f32)
                nc.vector.memset(wu[:], 0.0)
                wups = wup.tile([P, 512], f32)
                for _ in range(12):
                    nc.tensor.matmul(wups[:], wu[:, 0:P], wu[:],
                                     start=True, stop=True)

            # ---------------- Phase 1: transpose x, router, gates ----------
            with (
                tc.tile_pool(name="wcat_pool", bufs=1) as wcat_pool,
                tc.tile_pool(name="xc_pool", bufs=3) as xcp,
                tc.tile_pool(name="x32_pool", bufs=3) as x32p,
                tc.tile_pool(name="hg_pool", bufs=8) as hgp,
                tc.tile_pool(name="gates_pool", bufs=2) as gp,
                tc.tile_pool(name="ph_psum", bufs=2, space="PSUM") as php,
                tc.tile_pool(name="pr_psum", bufs=2, space="PSUM") as prp,
                tc.tile_pool(name="tr_psum", bufs=3, space="PSUM") as ptp,
            ):
                # prefetch the first tile's x ahead of the 4MB wcat stream,
                # so the first transposes don't wait behind it (the v5 trace
                # showed the PE idle ~7us at start and starved again at
                # tt=1-2 while wcat drained).
                xc_pre = []
                for q in range(2):
                    t_ = xcp.tile([P, 1024], f32, tag="xc")
                    nc.sync.dma_start(t_, xs[0:P, q * 1024:(q + 1) * 1024])
                    xc_pre.append(t_)
                # wcat split into per-chunk DMAs (a single 4MB transfer
                # starved the first tile's loraA matmuls for ~11us).
                wcat_sb = wcat_pool.tile([P, DC, NCAT], f32r)
                wcat_r = wcat.rearrange("(kc p) n -> p kc n", p=P)
                for kc in range(0, DC, 4):
                    nc.sync.dma_start(wcat_sb[:, kc:kc + 4, :],
                                      wcat_r[:, kc:kc + 4, :])

                hgs = []
                for tt in range(TT):
                    ts = slice(tt * P, (tt + 1) * P)
                    # transpose x tile [128, 4096] -> xT[:, :, ts] in groups
                    # of 4 sharing one PSUM bank so each psum->SBUF drain is
                    # one wide CAST (DVE) plus an exact-fp32 side copy on the
                    # otherwise-idle scalar engine feeding the router
                    # matmuls, which run inside the group stream so the
                    # transposes cover their LDWEIGHTS.
                    pr = prp.tile([P, E], f32, tag="pr")
                    for q in range(4):
                        if tt == 0 and q < 2:
                            xc = xc_pre[q]
                        else:
                            xc = xcp.tile([P, 1024], f32, tag="xc")
                            nc.sync.dma_start(
                                xc[:], xs[ts, q * 1024:(q + 1) * 1024])
                        for g in range(2):
                            pt = ptp.tile([P, 4, P], f32, tag="pt")
                            for j in range(4):
                                nc.tensor.transpose(
                                    pt[:, j, :],
                                    xc[:, (g * 4 + j) * P:(g * 4 + j + 1) * P],
                                    ident[:])
                            kc0 = q * 8 + g * 4
                            nc.vector.tensor_copy(
                                xT[:, kc0:kc0 + 4, ts], pt[:])
                            x32 = x32p.tile([P, 4, P], f32, tag="x32")
                            nc.scalar.copy(x32[:], pt[:])
                            for j in range(4):
                                kc = kc0 + j
                                nc.tensor.matmul(pr[:], x32[:, j, :],
                                                 wrouter_sb[:, kc, :],
                                                 start=(kc == 0),
                                                 stop=(kc == DC - 1))
                    # loraA as one dense accumulation run (fp32r)
                    ph = php.tile([P, NCAT], f32, tag="ph")
                    for kc in range(DC):
                        nc.tensor.matmul(ph[:], xT[:, kc, ts],
                                         wcat_sb[:, kc, :],
                                         start=(kc == 0), stop=(kc == DC - 1))
                    lg_sb = gp.tile([P, E], f32, tag="lgsb")
                    nc.vector.tensor_add(lg_sb[:], pr[:], cbias_sb[:])
                    lg = lg_sb[:]
                    # top-2 renormalized softmax gates (x SCALING)
                    m1 = gp.tile([P, 1], f32, tag="m1")
                    nc.vector.reduce_max(m1[:], lg, axis=X)
                    negm1 = gp.tile([P, 1], f32, tag="negm1")
                    nc.scalar.mul(negm1[:], m1[:], -1.0)
                    e_sb = gp.tile([P, E], f32, tag="esb")
                    nc.scalar.activation(e_sb[:], lg, Exp, bias=negm1[:])
                    t1 = gp.tile([P, E], f32, tag="t1")
                    nc.vector.scalar_tensor_tensor(
                        t1[:], lg, m1[:], negbig[:], is_ge, mult)
                    masked = gp.tile([P, E], f32, tag="masked")
                    nc.vector.tensor_add(masked[:], lg, t1[:])
                    m2 = gp.tile([P, 1], f32, tag="m2")
                    nc.vector.reduce_max(m2[:], masked[:], axis=X)
                    g_sb = gp.tile([P, E], f32, tag="gsb")
                    dsum = gp.tile([P, 1], f32, tag="dsum")
                    nc.vector.scalar_tensor_tensor(
                        g_sb[:], lg, m2[:], e_sb[:], is_ge, mult,
                        accum_out=dsum[:])
                    dhalf = gp.tile([P, 1], f32, tag="dhalf")
                    nc.scalar.mul(dhalf[:], dsum[:], 1.0 / SCALING)
                    rinv = gp.tile([P, 1], f32, tag="rinv")
                    nc.vector.reciprocal(rinv[:], dhalf[:])
                    gates = gp.tile([P, E], f32, tag="gates")
                    nc.vector.tensor_scalar_mul(gates[:], g_sb[:], rinv[:])
                    # hg = h * gates (broadcast over r), straight from PSUM;
                    # buffered per tile so the transposes can run later
                    # without stalling the PE on this DVE chain.
                    # bf16 hg: fits SBUF, transposes at 1c/r; quantization
                    # only touches the small LoRA term (~7e-4 rel).
                    hg = hgp.tile([P, ER], bf16, tag="hg")
                    nc.vector.tensor_tensor(
                        hg[:].rearrange("p (e r) -> p e r", e=E),
                        ph[:].rearrange("p (e r) -> p e r", e=E),
                        gates[:, :, None].to_broadcast([P, E, R]),
                        mult)
                    hgs.append(hg)

                # deferred hgT transposes: by now every tile's gate chain is
                # done (except possibly the last), so the PE never idles.
                for tt in range(TT):
                    ts = slice(tt * P, (tt + 1) * P)
                    pt = ptp.tile([P, 2, P], bf16, tag="pt")
                    for j in range(HC):
                        nc.tensor.transpose(
                            pt[:, j, :], hgs[tt][:, j * P:(j + 1) * P],
                            identb[:])
                    nc.vector.tensor_copy(hgT[:, :, ts], pt[:])

            # ---------------- Phase 2: fused [xT; hgT] @ wbig + bias ------
            with (
                tc.tile_pool(name="outp", bufs=4) as outp,
                tc.tile_pool(name="bias_pool", bufs=2) as biasp,
                tc.tile_pool(name="po_psum", bufs=8, space="PSUM") as pop,
            ):
                for ot in range(OT):
                    osl = slice(ot * ON, (ot + 1) * ON)
                    bias_sb = biasp.tile([P, ON], bf16, tag="bias")
                    nc.sync.dma_start(bias_sb[:], bias[:, osl])
                    ptiles = [pop.tile([P, ON], f32, tag="po",
                                       name=f"po_{ot}_{tt}")
                              for tt in range(TT)]
                    for kc in range(KC):
                        wt = wst.tile([P, ON], f32r, tag="w32")
                        nc.sync.dma_start(
                            wt[:], wbig[kc * P:(kc + 1) * P, osl])
                        for tt in range(TT):
                            ts = slice(tt * P, (tt + 1) * P)
                            lhsT = (xT[:, kc, ts] if kc < DC
                                    else hgT[:, kc - DC, ts])
                            nc.tensor.matmul(
                                ptiles[tt][:], lhsT, wt[:],
                                start=(kc == 0), stop=(kc == KC - 1))
                    for tt in range(TT):
                        ts = slice(tt * P, (tt + 1) * P)
                        osb = outp.tile([P, ON], f32, tag="osb")
                        nc.vector.tensor_add(
                            osb[:], ptiles[tt][:], bias_sb[:])
                        nc.sync.dma_start(out[ts, osl], osb[:])

    nc.compile()
    return nc


def _get_compiled():
    global _compiled
    if _compiled is None:
        _compiled = _build()
    return _compiled


def kernel(**inputs):
    global LAST_RESULT
    from concourse.bass_utils import run_bass_kernel_spmd

    import ml_dtypes

    x = np.ascontiguousarray(np.asarray(inputs["x"], dtype=np.float32))
    base_w = np.asarray(inputs["base_w"], dtype=np.float32)
    base_b = np.asarray(inputs["base_b"], dtype=np.float32)
    router_w = np.asarray(inputs["router_w"], dtype=np.float32)
    router_b = np.asarray(inputs["router_b"], dtype=np.float32)
    lora_a = np.asarray(inputs["lora_a"], dtype=np.float32)
    lora_b = np.asarray(inputs["lora_b"], dtype=np.float32)
    top_k = int(np.asarray(inputs.get("top_k", 2)))
    assert top_k == 2, "kernel is specialized for top_k=2"

    xt = x.reshape(T, D)
    # wbig rows: base_w.T (4096) then W2 (256) with W2[e*R+r, o] = lora_b[e,o,r]
    w2 = np.ascontiguousarray(
        lora_b.transpose(0, 2, 1).reshape(ER, O).astype(np.float32))
    wbig = np.ascontiguousarray(
        np.concatenate([base_w.T, w2], axis=0).astype(np.float32))
    # wcat: A_cat columns [d, er]; router weights separate (fp32-exact path)
    acat = lora_a.reshape(ER, D)  # [er, d]
    wcat = np.ascontiguousarray(acat.T.astype(np.float32))
    wrouter = np.ascontiguousarray(router_w.T.astype(np.float32))
    bias_full = np.ascontiguousarray(
        np.broadcast_to(base_b, (P, O)).astype(ml_dtypes.bfloat16))
    cbias_full = np.ascontiguousarray(
        np.broadcast_to(router_b.astype(np.float32), (P, E)))

    nc = _get_compiled()
    in_maps = [
        {"xs": np.ascontiguousarray(xt[c * TC:(c + 1) * TC]),
         "wbig": wbig, "wcat": wcat, "wrouter": wrouter,
         "bias": bias_full, "cbias": cbias_full}
        for c in range(NCORES)
    ]
    res = run_bass_kernel_spmd(nc, in_maps, core_ids=list(range(NCORES)),
                               trace=TRACE)
    LAST_RESULT = res
    outp = np.concatenate(
        [res.results[c]["out"] for c in range(NCORES)], axis=0)
    return outp.reshape(B, S, O).astype(np.float32)



# revision 8
# speedup vs baseline: 1.0877x; 1.0877x over previous
"""MoE-routing LoRA linear for Trainium2, SPMD over 8 NeuronCores.

out = x @ base_w.T + base_b + 2.0 * lora_out, where lora_out routes each
token through its top-2 (of 8) LoRA experts with renormalized softmax gates.

Strategy: data-parallel over tokens (1024 tokens/core), weights replicated.

v7 (trace-driven rewrite of the v6 673us baseline):
- Router matmuls (56us of PE in v6: 512 tiny fp32 4c/r matmuls) are fused
  into the loraA contraction: wcat gains 8 columns (router_w.T), so logits
  cost 8 extra cycles per accumulation step (~1us total). Logits become
  fp32r-precision (~5e-4 noise): ~10 near-tied tokens may flip their #2/#3
  expert vs the fp32 reference, bounded to <=4e-3 L2 (gate is 2e-2).
- Phase 2 flipped to output-transposed form: stationary = weight tiles
  (bf16, halves LDWEIGHTS vs fp32r's measured 187ns), moving = xT (bf16,
  [128,512] at 1 cycle/row). outT[o,t] drains via the scalar engine
  (Identity + per-partition bias AP) and is un-transposed on the host.
  bf16 x+w adds ~1.6e-3 L2 (compiler rejects bf16 x fp32r mixing, so both
  operands are bf16).
- Phase 1 transposes x once in fp32r (1.5c/r), drains each PSUM group
  twice: fp32r copy into a rotating per-tile xtr (feeds loraA+router at
  fp32r precision) on GpSimd, and a bf16 cast into the persistent xT on
  DVE. hgT transposes are emitted with a one-tile lag (tile 7's inside
  phase 2's first output column) so the PE never waits on a gate chain.
"""

import numpy as np

P = 128
B, S, D, O, E, R = 4, 2048, 4096, 4096, 8, 32
T = B * S            # 8192 tokens
NCORES = 8
TC = T // NCORES     # 1024 tokens per core
TT = TC // P         # 8 token tiles per core
DC = D // P          # 32 contraction chunks for x
ER = E * R           # 256
EA = ER + E          # 264: loraA columns + fused router columns
HC = ER // P         # 2 contraction chunks for hg
OC = O // P          # 32 output chunks (outT partition tiles)
HTC = TC // 2        # 512: half the tokens (one PSUM bank at fp32)
SCALING = 64.0 / 32.0

TRACE = False        # test harness sets kernel.TRACE = True for profiling
LAST_RESULT = None   # BassKernelResults of the last run (for exec_time_ns)

_compiled = None


def _build():
    import concourse.mybir as mybir
    import concourse.tile as tile
    from concourse import bacc
    from concourse.masks import make_identity

    f32 = mybir.dt.float32
    f32r = mybir.dt.float32r
    bf16 = mybir.dt.bfloat16
    X = mybir.AxisListType.X
    mult = mybir.AluOpType.mult
    is_ge = mybir.AluOpType.is_ge
    Exp = mybir.ActivationFunctionType.Exp

    nc = bacc.Bacc("TRN2", target_bir_lowering=False, debug=False,
                   num_devices=NCORES)

    # fp32r-declared x: DMA delivers raw fp32 bits; rounding happens in the
    # PE datapath (transpose rounds to ~tf32, verified on HW).
    xs = nc.dram_tensor("xs", [TC, D], f32r, kind="ExternalInput").ap()
    # wbase[oc, p, kc, f] = base_w[oc*P+f, kc*P+p], bf16 stationary tiles
    wbase = nc.dram_tensor("wbase", [OC, P, DC, P], bf16,
                           kind="ExternalInput").ap()
    # w2t[oc, p, j, f] = lora_b[e, oc*P+f, r] with j*P+p = e*R+r
    w2t = nc.dram_tensor("w2t", [OC, P, HC, P], bf16,
                         kind="ExternalInput").ap()
    # wcat[d, 0:256] = lora_a (A_cat^T), wcat[d, 256:264] = router_w.T
    wcat = nc.dram_tensor("wcat", [D, EA], f32r, kind="ExternalInput").ap()
    biasc = nc.dram_tensor("biasc", [P, OC], f32, kind="ExternalInput").ap()
    cbias = nc.dram_tensor("cbias", [P, E], f32, kind="ExternalInput").ap()
    out = nc.dram_tensor("out", [O, TC], f32, kind="ExternalOutput").ap()

    with tile.TileContext(nc) as tc:
        with (
            tc.tile_pool(name="persist", bufs=1) as persist,
            tc.tile_pool(name="consts", bufs=1) as consts,
            tc.tile_pool(name="wstream", bufs=6) as wst,
            tc.tile_pool(name="w2stream", bufs=2) as w2st,
            tc.tile_pool(name="hg_pool", bufs=8) as hgp,
            tc.tile_pool(name="tr2_psum", bufs=2, space="PSUM") as pt2p,
        ):
            xTb = persist.tile([P, DC, TC], bf16)    # bf16 x^T, phase-2 moving
            hgT = persist.tile([P, HC, TC], bf16)    # gated h transposed
            ident = consts.tile([P, P], f32)
            make_identity(nc, ident[:])
            identr = consts.tile([P, P], f32r)
            nc.vector.tensor_copy(identr[:], ident[:])
            identb = consts.tile([P, P], bf16)
            nc.vector.tensor_copy(identb[:], ident[:])
            cbias_sb = consts.tile([P, E], f32)
            nc.sync.dma_start(cbias_sb[:], cbias)
            biasc_sb = consts.tile([P, OC], f32)
            nc.sync.dma_start(biasc_sb[:], biasc)
            negbig = consts.tile([P, E], f32)
            nc.vector.memset(negbig[:], -1e30)

            # PE warm-up: ~2.5us of dense matmuls so the HAM clock gate opens
            with (
                tc.tile_pool(name="wu_pool", bufs=1) as wupl,
                tc.tile_pool(name="wu_psum", bufs=1, space="PSUM") as wup,
            ):
                wu = wupl.tile([P, 512], f32)
                nc.vector.memset(wu[:], 0.0)
                wups = wup.tile([P, 512], f32)
                for _ in range(12):
                    nc.tensor.matmul(wups[:], wu[:, 0:P], wu[:],
                                     start=True, stop=True)

            # ---------------- Phase 1: transpose x, loraA+router, gates ----
            with (
                tc.tile_pool(name="wcat_pool", bufs=1) as wcat_pool,
                tc.tile_pool(name="xc_pool", bufs=3) as xcp,
                tc.tile_pool(name="xtr_pool", bufs=2) as xtrp,
                tc.tile_pool(name="gates_pool", bufs=2) as gp,
                tc.tile_pool(name="ph_psum", bufs=2, space="PSUM") as php,
                tc.tile_pool(name="tr_psum", bufs=3, space="PSUM") as ptp,
            ):
                # prefetch the first tile's x ahead of the 4.3MB wcat stream
                xc_pre = []
                for q in range(2):
                    t_ = xcp.tile([P, 1024], f32r, tag="xc")
                    nc.sync.dma_start(t_, xs[0:P, q * 1024:(q + 1) * 1024])
                    xc_pre.append(t_)
                wcat_sb = wcat_pool.tile([P, DC, EA], f32r)
                wcat_r = wcat.rearrange("(kc p) n -> p kc n", p=P)
                for kc in range(0, DC, 4):
                    nc.sync.dma_start(wcat_sb[:, kc:kc + 4, :],
                                      wcat_r[:, kc:kc + 4, :])

                hgs = []

                def emit_hgT(tt):
                    ts_ = slice(tt * P, (tt + 1) * P)
                    pt2 = pt2p.tile([P, HC, P], bf16, tag="pt2")
                    for j in range(HC):
                        nc.tensor.transpose(
                            pt2[:, j, :], hgs[tt][:, j * P:(j + 1) * P],
                            identb[:])
                    nc.vector.tensor_copy(hgT[:, :, ts_], pt2[:])

                for tt in range(TT):
                    ts = slice(tt * P, (tt + 1) * P)
                    xtr = xtrp.tile([P, DC, P], f32r, tag="xtr")
                    for q in range(4):
                        if tt == 0 and q < 2:
                            xc = xc_pre[q]
                        else:
                            xc = xcp.tile([P, 1024], f32r, tag="xc")
                            nc.sync.dma_start(
                                xc[:], xs[ts, q * 1024:(q + 1) * 1024])
                        for g in range(2):
                            pt = ptp.tile([P, 4, P], f32r, tag="pt")
                            for j in range(4):
                                nc.tensor.transpose(
                                    pt[:, j, :],
                                    xc[:, (g * 4 + j) * P:(g * 4 + j + 1) * P],
                                    identr[:])
                            kc0 = q * 8 + g * 4
                            # raw fp32r drain for the loraA/router stationary
                            # (GpSimd cannot access PSUM, so DVE drains and
                            # the idle GpSimd does the SBUF->SBUF bf16 cast)
                            nc.vector.tensor_copy(xtr[:, kc0:kc0 + 4, :],
                                                  pt[:])
                            nc.gpsimd.tensor_copy(xTb[:, kc0:kc0 + 4, ts],
                                                  xtr[:, kc0:kc0 + 4, :])
                    # loraA + fused router as one accumulation run (fp32r)
                    ph = php.tile([P, EA], f32, tag="ph")
                    for kc in range(DC):
                        nc.tensor.matmul(ph[:], xtr[:, kc, :],
                                         wcat_sb[:, kc, :],
                                         start=(kc == 0), stop=(kc == DC - 1))
                    # top-2 renormalized softmax gates (x SCALING)
                    lg_sb = gp.tile([P, E], f32, tag="lgsb")
                    nc.vector.tensor_add(lg_sb[:], ph[:, ER:EA], cbias_sb[:])
                    lg = lg_sb[:]
                    m1 = gp.tile([P, 1], f32, tag="m1")
                    nc.vector.reduce_max(m1[:], lg, axis=X)
                    negm1 = gp.tile([P, 1], f32, tag="negm1")
                    nc.scalar.mul(negm1[:], m1[:], -1.0)
                    e_sb = gp.tile([P, E], f32, tag="esb")
                    nc.scalar.activation(e_sb[:], lg, Exp, bias=negm1[:])
                    t1 = gp.tile([P, E], f32, tag="t1")
                    nc.vector.scalar_tensor_tensor(
                        t1[:], lg, m1[:], negbig[:], is_ge, mult)
                    masked = gp.tile([P, E], f32, tag="masked")
                    nc.vector.tensor_add(masked[:], lg, t1[:])
                    m2 = gp.tile([P, 1], f32, tag="m2")
                    nc.vector.reduce_max(m2[:], masked[:], axis=X)
                    g_sb = gp.tile([P, E], f32, tag="gsb")
                    dsum = gp.tile([P, 1], f32, tag="dsum")
                    nc.vector.scalar_tensor_tensor(
                        g_sb[:], lg, m2[:], e_sb[:], is_ge, mult,
                        accum_out=dsum[:])
                    dhalf = gp.tile([P, 1], f32, tag="dhalf")
                    nc.scalar.mul(dhalf[:], dsum[:], 1.0 / SCALING)
                    rinv = gp.tile([P, 1], f32, tag="rinv")
                    nc.vector.reciprocal(rinv[:], dhalf[:])
                    gates = gp.tile([P, E], f32, tag="gates")
                    nc.vector.tensor_scalar_mul(gates[:], g_sb[:], rinv[:])
                    # hg = h * gates (broadcast over r), straight from PSUM
                    hg = hgp.tile([P, ER], bf16, tag="hg")
                    nc.vector.tensor_tensor(
                        hg[:].rearrange("p (e r) -> p e r", e=E),
                        ph[:, 0:ER].rearrange("p (e r) -> p e r", e=E),
                        gates[:, :, None].to_broadcast([P, E, R]),
                        mult)
                    hgs.append(hg)
                    # one-tile-lag hgT transpose: tile tt-1's gates are done
                    if tt >= 1:
                        emit_hgT(tt - 1)

            # ---------------- Phase 2: outT = wbig^T @ [xTb; hgT] + bias ---
            with (
                tc.tile_pool(name="outp", bufs=4) as outp,
                tc.tile_pool(name="po_psum", bufs=3, space="PSUM") as pop,
            ):
                for oc in range(OC):
                    poa = pop.tile([P, HTC], f32, tag="poa")
                    pob = pop.tile([P, HTC], f32, tag="pob")
                    wtg = None
                    for kc in range(DC):
                        if kc % 4 == 0:
                            wtg = wst.tile([P, 4, P], bf16, tag="wg")
                            nc.sync.dma_start(wtg[:],
                                              wbase[oc, :, kc:kc + 4, :])
                        st = wtg[:, kc % 4, :]
                        nc.tensor.matmul(poa[:], st, xTb[:, kc, 0:HTC],
                                         start=(kc == 0), stop=False)
                        nc.tensor.matmul(pob[:], st, xTb[:, kc, HTC:TC],
                                         start=(kc == 0), stop=False)
                    if oc == 0:
                        # tile 7's gate chain finished ~14us ago; no PE wait
                        emit_hgT(TT - 1)
                    w2g = w2st.tile([P, HC, P], bf16, tag="w2g")
                    nc.sync.dma_start(w2g[:], w2t[oc])
                    for j in range(HC):
                        nc.tensor.matmul(poa[:], w2g[:, j, :],
                                         hgT[:, j, 0:HTC],
                                         start=False, stop=(j == HC - 1))
                        nc.tensor.matmul(pob[:], w2g[:, j, :],
                                         hgT[:, j, HTC:TC],
                                         start=False, stop=(j == HC - 1))
                    osl = slice(oc * P, (oc + 1) * P)
                    osa = outp.tile([P, HTC], f32, tag="osa")
                    nc.scalar.add(osa[:], poa[:], biasc_sb[:, oc:oc + 1])
                    nc.sync.dma_start(out[osl, 0:HTC], osa[:])
                    osb = outp.tile([P, HTC], f32, tag="osb")
                    nc.scalar.add(osb[:], pob[:], biasc_sb[:, oc:oc + 1])
                    nc.sync.dma_start(out[osl, HTC:TC], osb[:])

    nc.compile()
    return nc


def _get_compiled():
    global _compiled
    if _compiled is None:
        _compiled = _build()
    return _compiled


def kernel(**inputs):
    global LAST_RESULT
    from concourse.bass_utils import run_bass_kernel_spmd

    import ml_dtypes

    bf = ml_dtypes.bfloat16
    x = np.ascontiguousarray(np.asarray(inputs["x"], dtype=np.float32))
    base_w = np.asarray(inputs["base_w"], dtype=np.float32)
    base_b = np.asarray(inputs["base_b"], dtype=np.float32)
    router_w = np.asarray(inputs["router_w"], dtype=np.float32)
    router_b = np.asarray(inputs["router_b"], dtype=np.float32)
    lora_a = np.asarray(inputs["lora_a"], dtype=np.float32)
    lora_b = np.asarray(inputs["lora_b"], dtype=np.float32)
    top_k = int(np.asarray(inputs.get("top_k", 2)))
    assert top_k == 2, "kernel is specialized for top_k=2"

    xt = x.reshape(T, D)
    # stationary weight tiles: wbase[oc, p, kc, f] = base_w.T[kc*P+p, oc*P+f]
    wbase = np.ascontiguousarray(
        base_w.T.reshape(DC, P, OC, P).transpose(2, 1, 0, 3).astype(bf))
    w2 = lora_b.transpose(0, 2, 1).reshape(ER, O)
    w2t = np.ascontiguousarray(
        w2.reshape(HC, P, OC, P).transpose(2, 1, 0, 3).astype(bf))
    acat = lora_a.reshape(ER, D)
    wcat = np.ascontiguousarray(np.concatenate(
        [acat.T, router_w.T], axis=1).astype(np.float32))
    biasc = np.ascontiguousarray(base_b.reshape(OC, P).T.astype(np.float32))
    cbias = np.ascontiguousarray(
        np.broadcast_to(router_b.astype(np.float32), (P, E)))

    nc = _get_compiled()
    in_maps = [
        {"xs": np.ascontiguousarray(xt[c * TC:(c + 1) * TC]),
         "wbase": wbase, "w2t": w2t, "wcat": wcat,
         "biasc": biasc, "cbias": cbias}
        for c in range(NCORES)
    ]
    res = run_bass_kernel_spmd(nc, in_maps, core_ids=list(range(NCORES)),
                               trace=TRACE)
    LAST_RESULT = res
    outp = np.empty((T, O), dtype=np.float32)
    for c in range(NCORES):
        outp[c * TC:(c + 1) * TC] = res.results[c]["out"].T
    return outp.reshape(B, S, O)
